# revision 11
# baseline (speedup 1.0000x reference)
"""Chamfer-with-normals loss kernel for Trainium2 (Bass/Tile), 8 NeuronCores.

Math (per batch item, N=4096 points):
    d[i,j] = ||ap_i - bp_j||^2 + w*(1 - <bn_i, an_j>)
           = aa[i] + bb[j] - 2<ap_i,bp_j> - w<bn_i,an_j> + w
    loss   = (sum_b [ sum_i min_j d + sum_j min_i d ]) / B

Sharding: data-parallel over batch B=8, one batch item per core. Each core
computes its 4096x4096 distance matrix tile-by-tile fully on-chip (PSUM),
reduces to a scalar partial; host sums the 8 partials.

The whole d matrix (minus the constant +w, added on host) is produced by a
single K=128 fp32 matmul per tile, with the 4 feature groups at partitions
{0:3, 32:35, 64:67, 96:99} and exact-zero rows elsewhere (K is free on the
PE; aligned starts satisfy the compute-engine SBUF partition rule):
    rows 0:3   sqrt(2)*a_pts x -sqrt(2)*b_pts -> -2<ap_i,bp_j>
    rows 32:35 sqrt(w)*b_nrm x -sqrt(w)*a_nrm -> -w<bn_i,an_j>
    rows 64:67 a_pts^2       x  1             -> aa[i]
    rows 96:99 1             x  b_pts^2       -> bb[j]

Measured DVE/ACT costs (this hardware, FD=2048 [128,FD] tiles):
    fp32 tensor_reduce   ~4.2us   fp32 tensor_tensor ~3.4us   (1x + drain)
    fp16 tensor_tensor   ~1.3us   (2x mode, no drain penalty)
    fp16 tensor_reduce   ~0.25us  (!! ~8 elem/lane/cycle fast path)
    ACT copy             ~0.6us per FD=512 chunk (one 2048 op pays ~2x/elem)
The fp32-from-PSUM baseline (reduce + TT per tile) was ~7.4us/tile. Swept
alternatives that LOST to this config (interleaved A/B on hardware):
act_chunk 256/1024/2048, col_chunk in sub-chunks, sharing the fp32->fp16
conversion with the DVE (tensor_copy from PSUM pays the fp32 1x + drain).

Per (row-tile, col-group) tile of [128, 2048] (4 PSUM banks, x2 buffered):
    PE:  4 fp32 matmuls (N=512 each), ~0.9us warm (a junk-MM burst at loop
         start ramps the HAM clock gate 1.2->2.4GHz; per-tile gaps are short
         enough to stay warm)
    ACT: copies the PSUM tile to an fp16 SBUF tile in FD=512 chunks (~2.3us)
    DVE: one fp16 2x tensor_tensor min into colmin[:,g,:] plus one fp16
         fast-path tensor_reduce row-min into rm16[:,g,mt]  (~1.6us)
ACT is the bottleneck engine at ~2.3us/tile; DVE ~1.6us; PE ~0.9us.
Final: rm16 pair-min over groups + summed; colmin converted to fp32,
partition-axis min via PE transpose + DVE reduce; partition sum via one more
PE transpose; scalar DMA'd out. Host sums the 8 per-core partials.
fp16 quantization of d (values O(10), mins O(0.01)) costs ~1e-5 rel on the
final loss.
"""

import numpy as np

import concourse.bacc as bacc
import concourse.bass as bass
import concourse.tile as tile
from concourse import mybir
from concourse.masks import make_identity

B = 8
C = 6
N = 4096
W = 0.001
P = 128
BIG = 1.0e30  # +inf surrogate (keeps finiteness checks happy)

F32 = mybir.dt.float32
F16 = mybir.dt.float16
F16BIG = 60000.0  # fp16-finite +inf surrogate; d values are O(100)
MIN = mybir.AluOpType.min
ADD = mybir.AluOpType.add
MULT = mybir.AluOpType.mult


def build_nc(n=N, g_cols=2048, act_chunk=512, col_chunk=2048, repeat=1,
             warm_mms=12, dve_share=0, do_act=True, do_tt=True, do_red=True):
    """Build the single-core Bass program (SPMD across 8 cores).

    repeat>1 re-runs the (idempotent) main loop that many times inside a
    device-side For_i — used to measure true HW kernel time by wallclock
    differencing across the axon tunnel.
    """
    assert n % P == 0 and g_cols % 512 == 0 and n % g_cols == 0
    assert g_cols % act_chunk == 0 and g_cols % col_chunk == 0
    n_mt = n // P          # row tiles
    n_g = n // g_cols      # column groups

    nc = bacc.Bacc(trn_type="TRN2", debug=False, enable_partition_id=False)
    a_dram = nc.dram_tensor("a_local", [C, n], F32, kind="ExternalInput").ap()
    b_dram = nc.dram_tensor("b_local", [C, n], F32, kind="ExternalInput").ap()
    out_dram = nc.dram_tensor("out", [1, 1], F32, kind="ExternalOutput").ap()
    # Never written -> the runtime's zero-initialized output buffer doubles as
    # a zero source, so the dead-row zero-fill is DMA work instead of ~17us of
    # DVE memsets.
    zeros_dram = nc.dram_tensor("zeros", [32, n], F32, kind="ExternalOutput").ap()

    with tile.TileContext(nc) as tc:
        with (
            tc.tile_pool(name="singles", bufs=1) as singles,
            tc.tile_pool(name="dtiles", bufs=3) as dpool,
        ):
            # ---------------- setup ----------------
            # K=128 matmul with zero rows: feature groups live at partitions
            # {0:3, 32:35, 64:67, 96:99} (compute-legal SBUF starts), all
            # other rows are exact zeros. Matmul cost is K-independent, so
            # this costs nothing on the PE and avoids any partition-odd
            # assembly DMAs (whose many semaphores overflow the per-
            # instruction HW wait-command limit).
            #   rows 0:3   sqrt(2)*a_pts  x  -sqrt(2)*b_pts  -> -2<ap,bp>
            #   rows 32:35 sqrt(w)*b_nrm  x  -sqrt(w)*a_nrm  -> -w<bn,an>
            #   rows 64:67 a_pts^2        x  1               -> aa[i]
            #   rows 96:99 1              x  b_pts^2         -> bb[j]
            # Scales split as +/-sqrt so every live row's LAST writer is a
            # compute engine: xt deps = {DVE}, yt deps = {GPSIMD} only.
            s2 = float(np.sqrt(2.0))
            sw = float(np.sqrt(W))
            xt = singles.tile([P, n], F32)  # lhsT rows (a-side features)
            yt = singles.tile([P, n], F32)  # rhs rows  (b-side features)

            # zero everything first (dead rows must be exact 0; live rows get
            # overwritten by the DMAs + fills below, WAW deps keep the order)
            for t in (xt, yt):
                for p0 in (0, 32, 64, 96):
                    nc.sync.dma_start(out=t[p0:p0 + 32, :], in_=zeros_dram[:, :])

            # inputs land in f32 staging tiles (same partitions as their
            # destination rows); compute fills below write the operand tiles
            stage_a = singles.tile([P, n], F32)
            stage_b = singles.tile([P, n], F32)
            nc.sync.dma_start(out=stage_a[0:3, :], in_=a_dram[0:3, :])
            nc.sync.dma_start(out=stage_a[32:35, :], in_=b_dram[3:6, :])
            nc.sync.dma_start(out=stage_a[64:67, :], in_=a_dram[0:3, :])
            nc.sync.dma_start(out=stage_b[0:3, :], in_=b_dram[0:3, :])
            nc.sync.dma_start(out=stage_b[32:35, :], in_=a_dram[3:6, :])
            nc.sync.dma_start(out=stage_b[96:99, :], in_=b_dram[0:3, :])

            # xt transforms on DVE, yt on GPSIMD, except the yt square
            # (2-input ops are ~2x slower on GPSIMD) which goes to DVE
            nc.vector.tensor_scalar(
                out=xt[0:3, :], in0=stage_a[0:3, :], scalar1=s2, scalar2=None, op0=MULT)
            nc.vector.tensor_scalar(
                out=xt[32:35, :], in0=stage_a[32:35, :], scalar1=sw, scalar2=None, op0=MULT)
            nc.vector.tensor_tensor(
                out=xt[64:67, :], in0=stage_a[64:67, :], in1=stage_a[64:67, :], op=MULT)
            nc.gpsimd.memset(xt[96:99, :], 1.0)
            nc.gpsimd.tensor_scalar(
                out=yt[0:3, :], in0=stage_b[0:3, :], scalar1=-s2, scalar2=None, op0=MULT)
            nc.gpsimd.tensor_scalar(
                out=yt[32:35, :], in0=stage_b[32:35, :], scalar1=-sw, scalar2=None, op0=MULT)
            nc.vector.tensor_tensor(
                out=yt[96:99, :], in0=stage_b[96:99, :], in1=stage_b[96:99, :], op=MULT)
            nc.gpsimd.memset(yt[64:67, :], 1.0)

            # fp16 min accumulators. rm16[:, g, mt] holds row-tile mt's
            # per-row min over column group g (fp16 reduce has an ~8
            # elem/lane/cycle fast path); colmin[:, g, :] the running
            # per-column min over row tiles.
            rm16 = singles.tile([P, n_g, n_mt], F16)
            nc.vector.memset(rm16[:].rearrange("p a b -> p (a b)"), F16BIG)
            colmin = singles.tile([P, n_g, g_cols], F16)
            nc.vector.memset(colmin[:].rearrange("p a b -> p (a b)"), F16BIG)
            # static fp16 source for engine-ablation timing runs (do_act=False)
            dummy16 = singles.tile([P, g_cols], F16)
            nc.vector.memset(dummy16, F16BIG)

            # ---------------- main loop ----------------
            import contextlib
            rep_ctx = tc.For_i(0, repeat, 1) if repeat > 1 else contextlib.nullcontext()
            with tc.tile_pool(name="psum_d", bufs=2, space="PSUM") as pd_pool, rep_ctx:
                for mt in range(n_mt):
                    lhsT = xt[:, mt * P:(mt + 1) * P]
                    for g in range(n_g):
                        ps = pd_pool.tile([P, g_cols], F32, tag="ps")
                        if mt == 0 and g == 0:
                            # HAM warm-up: a back-to-back junk-MM burst (>4us
                            # PE-busy) ramps the PE clock gate 1.2->2.4GHz;
                            # the real q=0 matmul (start=True) overwrites the
                            # target bank, so this is idempotent. Later
                            # per-tile idle gaps are short enough to stay warm.
                            for _ in range(warm_mms):
                                nc.tensor.matmul(
                                    ps[:, 0:512], lhsT, yt[:, 0:512],
                                    start=True, stop=True,
                                )
                        for q in range(g_cols // 512):
                            j0 = g * g_cols + q * 512
                            nc.tensor.matmul(
                                ps[:, q * 512:(q + 1) * 512],
                                lhsT,
                                yt[:, j0:j0 + 512],
                                start=True, stop=True,
                            )
                        # otherwise-idle ACT engine converts d to fp16 SBUF
                        # (chunked: large-FD ACT ops pay a ~2x penalty)
                        if do_act:
                            dt16 = dpool.tile([P, g_cols], F16, tag="dt16")
                            for c0 in range(0, g_cols - dve_share, act_chunk):
                                cs = slice(c0, min(c0 + act_chunk, g_cols - dve_share))
                                nc.scalar.activation(
                                    out=dt16[:, cs], in_=ps[:, cs],
                                    func=mybir.ActivationFunctionType.Copy,
                                )
                            if dve_share:
                                # DVE helps with the conversion (it has slack
                                # vs the ACT): fp32-PSUM copy runs 1x
                                cs = slice(g_cols - dve_share, g_cols)
                                nc.vector.tensor_copy(dt16[:, cs], ps[:, cs])
                        else:
                            dt16 = dummy16
                        # row-min via the fp16 fast-path reduce
                        if do_red:
                            nc.vector.tensor_reduce(
                                out=rm16[:, g, mt:mt + 1], in_=dt16,
                                axis=mybir.AxisListType.X, op=MIN,
                            )
                        # col-min accumulate, fp16 2x
                        if do_tt:
                            for c0 in range(0, g_cols, col_chunk):
                                cs = slice(c0, c0 + col_chunk)
                                nc.vector.tensor_tensor(
                                    out=colmin[:, g, cs], in0=dt16[:, cs],
                                    in1=colmin[:, g, cs], op=MIN,
                                )

            # ---------------- final reduction ----------------
            identity = singles.tile([P, P], F32)
            make_identity(nc, identity)

            # row side: min across the n_g groups (pairwise TT on contiguous
            # [P, n_mt] slices), convert, then sum over rows in this
            # partition; fp16 min is exact.
            rmin16 = singles.tile([P, n_mt], F16)
            assert n_g == 2
            nc.vector.tensor_tensor(
                out=rmin16, in0=rm16[:, 0, :], in1=rm16[:, 1, :], op=MIN)
            rm32 = singles.tile([P, n_mt], F32)
            nc.vector.tensor_copy(rm32, rmin16)
            row_sum = singles.tile([P, 1], F32)
            nc.vector.tensor_reduce(out=row_sum, in_=rm32, axis=mybir.AxisListType.X, op=ADD)

            # col side: convert col-min accumulator to fp32 before the
            # PE-transpose tail (16-bit transpose into PSUM is the exotic
            # path; avoid), then partition-axis min via PE transpose
            colmin32 = singles.tile([P, n_g, g_cols], F32)
            nc.vector.tensor_copy(
                colmin32[:].rearrange("p a b -> p (a b)"),
                colmin[:].rearrange("p a b -> p (a b)"))
            n_chunks = n // P
            collector = singles.tile([P, n_chunks], F32)
            with tc.tile_pool(name="psum_t", bufs=4, space="PSUM") as pt_pool:
                cm_flat = colmin32[:].rearrange("p a b -> p (a b)")
                for t in range(n_chunks):
                    psT = pt_pool.tile([P, P], F32, tag="psT")
                    nc.tensor.transpose(psT, cm_flat[:, t * P:(t + 1) * P], identity)
                    nc.vector.tensor_reduce(
                        out=collector[:, t:t + 1], in_=psT,
                        axis=mybir.AxisListType.X, op=MIN,
                    )

                col_sum = singles.tile([P, 1], F32)
                nc.vector.tensor_reduce(out=col_sum, in_=collector, axis=mybir.AxisListType.X, op=ADD)

                total_p = singles.tile([P, 1], F32)
                nc.vector.tensor_tensor(out=total_p, in0=row_sum, in1=col_sum, op=ADD)

                psF = pt_pool.tile([1, P], F32, tag="psF")
                nc.tensor.transpose(psF, total_p, identity)
                loss_sb = singles.tile([1, 1], F32)
                nc.vector.tensor_reduce(out=loss_sb, in_=psF, axis=mybir.AxisListType.X, op=ADD)

            nc.sync.dma_start(out=out_dram[:, :], in_=loss_sb[0:1, 0:1])

    nc.compile()  # bacc passes: split multi-waits (TRN2: 1 wait/instruction), etc.
    return nc


_NC_CACHE = {}


def _get_nc():
    if "nc" not in _NC_CACHE:
        _NC_CACHE["nc"] = build_nc()
    return _NC_CACHE["nc"]


def kernel(a: np.ndarray, b: np.ndarray) -> np.ndarray:
    """Full inputs a, b: [B, 6, N] float32 -> scalar float32 loss."""
    from concourse.bass_utils import run_bass_kernel_spmd

    a = np.ascontiguousarray(np.asarray(a), dtype=np.float32)
    b = np.ascontiguousarray(np.asarray(b), dtype=np.float32)
    assert a.shape == (B, C, N) and b.shape == (B, C, N)

    nc = _get_nc()
    in_maps = [{"a_local": a[c], "b_local": b[c]} for c in range(B)]
    res = run_bass_kernel_spmd(nc, in_maps, core_ids=list(range(B)))
    partials = [float(r["out"][0, 0]) for r in res.results]
    # each core's partial omits the +w constant inside d: min_j(core+w) = w + min_j(core),
    # contributing 2*N*w per batch item; /B at the end.
    total = (sum(partials)) / B + 2 * N * W
    return np.asarray(total, dtype=np.float32)


# revision 13
# speedup vs baseline: 1.2161x; 1.2161x over previous
"""Chamfer-with-normals loss kernel for Trainium2 (Bass/Tile), 8 NeuronCores.

Math (per batch item, N=4096 points):
    d[i,j] = ||ap_i - bp_j||^2 + w*(1 - <bn_i, an_j>)
           = aa[i] + bb[j] - 2<ap_i,bp_j> - w<bn_i,an_j> + w
    loss   = (sum_b [ sum_i min_j d + sum_j min_i d ]) / B

Sharding: data-parallel over batch B=8, one batch item per core. Each core
computes its 4096x4096 distance matrix tile-by-tile fully on-chip (PSUM),
reduces to a scalar partial; host sums the 8 partials.

The whole d matrix (minus the constant +w, added on host) is produced by a
single K=128 fp32 matmul per tile, with the 4 feature groups at partitions
{0:3, 32:35, 64:67, 96:99} and exact-zero rows elsewhere (K is free on the
PE; aligned starts satisfy the compute-engine SBUF partition rule):
    rows 0:3   sqrt(2)*a_pts x -sqrt(2)*b_pts -> -2<ap_i,bp_j>
    rows 32:35 sqrt(w)*b_nrm x -sqrt(w)*a_nrm -> -w<bn_i,an_j>
    rows 64:67 a_pts^2       x  1             -> aa[i]
    rows 96:99 1             x  b_pts^2       -> bb[j]

Measured DVE/ACT costs (this hardware, FD=2048 [128,FD] tiles):
    fp32 tensor_reduce   ~4.2us   fp32 tensor_tensor ~3.4us   (1x + drain)
    fp16 tensor_tensor   ~1.3us   (2x mode, no drain penalty)
    fp16 tensor_reduce   ~0.25us  (!! ~8 elem/lane/cycle fast path)
    ACT copy             ~0.6us per FD=512 chunk (one 2048 op pays ~2x/elem)
The fp32-from-PSUM baseline (reduce + TT per tile) was ~7.4us/tile. Swept
alternatives that LOST to this config (interleaved A/B on hardware):
act_chunk 256/1024/2048, col_chunk in sub-chunks, sharing the fp32->fp16
conversion with the DVE (tensor_copy from PSUM pays the fp32 1x + drain).

Per (row-tile, col-group) tile of [128, 2048] (4 PSUM banks, x2 buffered):
    PE:  4 fp32 matmuls (N=512 each), ~0.9us warm (a junk-MM burst at loop
         start ramps the HAM clock gate 1.2->2.4GHz; per-tile gaps are short
         enough to stay warm)
    ACT: copies the PSUM tile to an fp16 SBUF tile in FD=512 chunks (~2.3us)
    DVE: one fp16 2x tensor_tensor min into colmin[:,g,:] plus one fp16
         fast-path tensor_reduce row-min into rm16[:,g,mt]  (~1.6us)
ACT is the bottleneck engine at ~2.3us/tile; DVE ~1.6us; PE ~0.9us.
Final: rm16 pair-min over groups + summed; colmin converted to fp32,
partition-axis min via PE transpose + DVE reduce; partition sum via one more
PE transpose; scalar DMA'd out. Host sums the 8 per-core partials.
fp16 quantization of d (values O(10), mins O(0.01)) costs ~1e-5 rel on the
final loss.
"""

import numpy as np

import concourse.bacc as bacc
import concourse.bass as bass
import concourse.tile as tile
from concourse import mybir
from concourse.masks import make_identity

B = 8
C = 6
N = 4096
W = 0.001
P = 128
BIG = 1.0e30  # +inf surrogate (keeps finiteness checks happy)

F32 = mybir.dt.float32
F16 = mybir.dt.float16
F16BIG = 60000.0  # fp16-finite +inf surrogate; d values are O(100)
MIN = mybir.AluOpType.min
ADD = mybir.AluOpType.add
MULT = mybir.AluOpType.mult


def build_nc(n=N, g_cols=2048, act_chunk=512, col_chunk=2048, repeat=1,
             warm_mms=12, dve_share=0, dbufs=3,
             do_act=True, do_tt=True, do_red=True):
    """Build the single-core Bass program (SPMD across 8 cores).

    repeat>1 re-runs the (idempotent) main loop that many times inside a
    device-side For_i — used to measure true HW kernel time by wallclock
    differencing across the axon tunnel.
    """
    assert n % P == 0 and g_cols % 512 == 0 and n % g_cols == 0
    assert g_cols % col_chunk == 0
    n_mt = n // P          # row tiles
    n_g = n // g_cols      # column groups

    nc = bacc.Bacc(trn_type="TRN2", debug=False, enable_partition_id=False)
    a_dram = nc.dram_tensor("a_local", [C, n], F32, kind="ExternalInput").ap()
    b_dram = nc.dram_tensor("b_local", [C, n], F32, kind="ExternalInput").ap()
    out_dram = nc.dram_tensor("out", [1, 1], F32, kind="ExternalOutput").ap()
    # Never written -> the runtime's zero-initialized output buffer doubles as
    # a zero source, so the dead-row zero-fill is DMA work instead of ~17us of
    # DVE memsets.
    zeros_dram = nc.dram_tensor("zeros", [32, n], F32, kind="ExternalOutput").ap()

    with tile.TileContext(nc) as tc:
        with (
            tc.tile_pool(name="singles", bufs=1) as singles,
            tc.tile_pool(name="dtiles", bufs=dbufs) as dpool,
        ):
            # ---------------- setup ----------------
            # K=128 matmul with zero rows: feature groups live at partitions
            # {0:3, 32:35, 64:67, 96:99} (compute-legal SBUF starts), all
            # other rows are exact zeros. Matmul cost is K-independent, so
            # this costs nothing on the PE and avoids any partition-odd
            # assembly DMAs (whose many semaphores overflow the per-
            # instruction HW wait-command limit).
            #   rows 0:3   sqrt(2)*a_pts  x  -sqrt(2)*b_pts  -> -2<ap,bp>
            #   rows 32:35 sqrt(w)*b_nrm  x  -sqrt(w)*a_nrm  -> -w<bn,an>
            #   rows 64:67 a_pts^2        x  1               -> aa[i]
            #   rows 96:99 1              x  b_pts^2         -> bb[j]
            # Scales split as +/-sqrt so every live row's LAST writer is a
            # compute engine: xt deps = {DVE}, yt deps = {GPSIMD} only.
            s2 = float(np.sqrt(2.0))
            sw = float(np.sqrt(W))
            xt = singles.tile([P, n], F32)  # lhsT rows (a-side features)
            yt = singles.tile([P, n], F32)  # rhs rows  (b-side features)

            # zero everything first (dead rows must be exact 0; live rows get
            # overwritten by the DMAs + fills below, WAW deps keep the order)
            for t in (xt, yt):
                for p0 in (0, 32, 64, 96):
                    nc.sync.dma_start(out=t[p0:p0 + 32, :], in_=zeros_dram[:, :])

            # inputs land in f32 staging tiles (same partitions as their
            # destination rows); compute fills below write the operand tiles
            stage_a = singles.tile([P, n], F32)
            stage_b = singles.tile([P, n], F32)
            nc.sync.dma_start(out=stage_a[0:3, :], in_=a_dram[0:3, :])
            nc.sync.dma_start(out=stage_a[32:35, :], in_=b_dram[3:6, :])
            nc.sync.dma_start(out=stage_a[64:67, :], in_=a_dram[0:3, :])
            nc.sync.dma_start(out=stage_b[0:3, :], in_=b_dram[0:3, :])
            nc.sync.dma_start(out=stage_b[32:35, :], in_=a_dram[3:6, :])
            nc.sync.dma_start(out=stage_b[96:99, :], in_=b_dram[0:3, :])

            # xt transforms on DVE, yt on GPSIMD, except the yt square
            # (2-input ops are ~2x slower on GPSIMD) which goes to DVE
            nc.vector.tensor_scalar(
                out=xt[0:3, :], in0=stage_a[0:3, :], scalar1=s2, scalar2=None, op0=MULT)
            nc.vector.tensor_scalar(
                out=xt[32:35, :], in0=stage_a[32:35, :], scalar1=sw, scalar2=None, op0=MULT)
            nc.vector.tensor_tensor(
                out=xt[64:67, :], in0=stage_a[64:67, :], in1=stage_a[64:67, :], op=MULT)
            nc.gpsimd.memset(xt[96:99, :], 1.0)
            nc.gpsimd.tensor_scalar(
                out=yt[0:3, :], in0=stage_b[0:3, :], scalar1=-s2, scalar2=None, op0=MULT)
            nc.gpsimd.tensor_scalar(
                out=yt[32:35, :], in0=stage_b[32:35, :], scalar1=-sw, scalar2=None, op0=MULT)
            nc.vector.tensor_tensor(
                out=yt[96:99, :], in0=stage_b[96:99, :], in1=stage_b[96:99, :], op=MULT)
            nc.gpsimd.memset(yt[64:67, :], 1.0)

            # fp16 min accumulators. rm16[:, g, mt] holds row-tile mt's
            # per-row min over column group g (fp16 reduce has an ~8
            # elem/lane/cycle fast path); colmin[:, g, :] the running
            # per-column min over row tiles.
            rm16 = singles.tile([P, n_g, n_mt], F16)
            nc.vector.memset(rm16[:].rearrange("p a b -> p (a b)"), F16BIG)
            colmin = singles.tile([P, n_g, g_cols], F16)
            nc.vector.memset(colmin[:].rearrange("p a b -> p (a b)"), F16BIG)
            # static fp16 source for engine-ablation timing runs (do_act=False)
            dummy16 = singles.tile([P, g_cols], F16)
            nc.vector.memset(dummy16, F16BIG)

            # ---------------- main loop ----------------
            import contextlib
            rep_ctx = tc.For_i(0, repeat, 1) if repeat > 1 else contextlib.nullcontext()
            with tc.tile_pool(name="psum_d", bufs=2, space="PSUM") as pd_pool, rep_ctx:
                for mt in range(n_mt):
                    lhsT = xt[:, mt * P:(mt + 1) * P]
                    for g in range(n_g):
                        ps = pd_pool.tile([P, g_cols], F32, tag="ps")
                        if mt == 0 and g == 0:
                            # HAM warm-up: a back-to-back junk-MM burst (>4us
                            # PE-busy) ramps the PE clock gate 1.2->2.4GHz;
                            # the real q=0 matmul (start=True) overwrites the
                            # target bank, so this is idempotent. Later
                            # per-tile idle gaps are short enough to stay warm.
                            for _ in range(warm_mms):
                                nc.tensor.matmul(
                                    ps[:, 0:512], lhsT, yt[:, 0:512],
                                    start=True, stop=True,
                                )
                        for q in range(g_cols // 512):
                            j0 = g * g_cols + q * 512
                            nc.tensor.matmul(
                                ps[:, q * 512:(q + 1) * 512],
                                lhsT,
                                yt[:, j0:j0 + 512],
                                start=True, stop=True,
                            )
                        # otherwise-idle ACT engine converts d to fp16 SBUF
                        # (chunked: large-FD ACT ops pay a ~2x penalty)
                        if do_act:
                            dt16 = dpool.tile([P, g_cols], F16, tag="dt16")
                            for c0 in range(0, g_cols - dve_share, act_chunk):
                                cs = slice(c0, min(c0 + act_chunk, g_cols - dve_share))
                                nc.scalar.activation(
                                    out=dt16[:, cs], in_=ps[:, cs],
                                    func=mybir.ActivationFunctionType.Copy,
                                )
                            if dve_share:
                                # DVE helps with the conversion (it has slack
                                # vs the ACT): fp32-PSUM copy runs 1x
                                cs = slice(g_cols - dve_share, g_cols)
                                nc.vector.tensor_copy(dt16[:, cs], ps[:, cs])
                        else:
                            dt16 = dummy16
                        # row-min via the fp16 fast-path reduce
                        if do_red:
                            nc.vector.tensor_reduce(
                                out=rm16[:, g, mt:mt + 1], in_=dt16,
                                axis=mybir.AxisListType.X, op=MIN,
                            )
                        # col-min accumulate, fp16 2x
                        if do_tt:
                            for c0 in range(0, g_cols, col_chunk):
                                cs = slice(c0, c0 + col_chunk)
                                nc.vector.tensor_tensor(
                                    out=colmin[:, g, cs], in0=dt16[:, cs],
                                    in1=colmin[:, g, cs], op=MIN,
                                )

            # ---------------- final reduction ----------------
            identity = singles.tile([P, P], F32)
            make_identity(nc, identity)

            # row side: min across the n_g groups (pairwise TT on contiguous
            # [P, n_mt] slices), convert, then sum over rows in this
            # partition; fp16 min is exact.
            rmin16 = singles.tile([P, n_mt], F16)
            assert n_g == 2
            nc.vector.tensor_tensor(
                out=rmin16, in0=rm16[:, 0, :], in1=rm16[:, 1, :], op=MIN)
            rm32 = singles.tile([P, n_mt], F32)
            nc.vector.tensor_copy(rm32, rmin16)
            row_sum = singles.tile([P, 1], F32)
            nc.vector.tensor_reduce(out=row_sum, in_=rm32, axis=mybir.AxisListType.X, op=ADD)

            # col side: convert col-min accumulator to fp32 before the
            # PE-transpose tail (16-bit transpose into PSUM is the exotic
            # path; avoid), then partition-axis min via PE transpose
            colmin32 = singles.tile([P, n_g, g_cols], F32)
            nc.vector.tensor_copy(
                colmin32[:].rearrange("p a b -> p (a b)"),
                colmin[:].rearrange("p a b -> p (a b)"))
            n_chunks = n // P
            collector = singles.tile([P, n_chunks], F32)
            with tc.tile_pool(name="psum_t", bufs=4, space="PSUM") as pt_pool:
                cm_flat = colmin32[:].rearrange("p a b -> p (a b)")
                for t in range(n_chunks):
                    psT = pt_pool.tile([P, P], F32, tag="psT")
                    nc.tensor.transpose(psT, cm_flat[:, t * P:(t + 1) * P], identity)
                    nc.vector.tensor_reduce(
                        out=collector[:, t:t + 1], in_=psT,
                        axis=mybir.AxisListType.X, op=MIN,
                    )

                col_sum = singles.tile([P, 1], F32)
                nc.vector.tensor_reduce(out=col_sum, in_=collector, axis=mybir.AxisListType.X, op=ADD)

                total_p = singles.tile([P, 1], F32)
                nc.vector.tensor_tensor(out=total_p, in0=row_sum, in1=col_sum, op=ADD)

                psF = pt_pool.tile([1, P], F32, tag="psF")
                nc.tensor.transpose(psF, total_p, identity)
                loss_sb = singles.tile([1, 1], F32)
                nc.vector.tensor_reduce(out=loss_sb, in_=psF, axis=mybir.AxisListType.X, op=ADD)

            nc.sync.dma_start(out=out_dram[:, :], in_=loss_sb[0:1, 0:1])

    nc.compile()  # bacc passes: split multi-waits (TRN2: 1 wait/instruction), etc.
    return nc


_NC_CACHE = {}


def _get_nc():
    if "nc" not in _NC_CACHE:
        _NC_CACHE["nc"] = build_nc()
    return _NC_CACHE["nc"]


def kernel(a: np.ndarray, b: np.ndarray) -> np.ndarray:
    """Full inputs a, b: [B, 6, N] float32 -> scalar float32 loss."""
    from concourse.bass_utils import run_bass_kernel_spmd

    a = np.ascontiguousarray(np.asarray(a), dtype=np.float32)
    b = np.ascontiguousarray(np.asarray(b), dtype=np.float32)
    assert a.shape == (B, C, N) and b.shape == (B, C, N)

    nc = _get_nc()
    in_maps = [{"a_local": a[c], "b_local": b[c]} for c in range(B)]
    res = run_bass_kernel_spmd(nc, in_maps, core_ids=list(range(B)))
    partials = [float(r["out"][0, 0]) for r in res.results]
    # each core's partial omits the +w constant inside d: min_j(core+w) = w + min_j(core),
    # contributing 2*N*w per batch item; /B at the end.
    total = (sum(partials)) / B + 2 * N * W
    return np.asarray(total, dtype=np.float32)
